# revision 81
# baseline (speedup 1.0000x reference)
"""Trainium2 Bass kernel for the CGC multi-task MoE routing problem.

Full-input contract: kernel(**inputs) takes the unsharded numpy inputs and
returns the full [T+1, B, E] float32 output.

Strategy: pure data-parallel over batch across 8 NeuronCores (weights
replicated, no collectives). Per core (B_loc = 1024):
  - 16 experts (12 task-specific + 4 shared), each a 2-layer ReLU MLP:
        layer 1 feature-major on TensorE:  hT[H,B] = relu(W1.T @ xT + b1)
        layer 2 batch-major:               s[B,E]  = relu(hT.T @ W2)
    (hT is exactly the lhsT layer 2 needs, so expert outputs land in
    [B, E] orientation with no PE transposes at all)
  - layer-1 contraction entirely via fp8(e4m3) DoubleRow matmuls at 0.5
    cycles/row (4 slices of 256 dims, 2 k-tiles each). The activations
    are split hi+lo (x ~= hi + lo, both e4m3, pre-scaled /4); the lo
    correction runs on only NSL_LO=3 of 4 slices — dropping slice 3's
    x-residual adds 0.9e-2 rel err in quadrature over the single-e4m3 W
    quantization error (1.90e-2 end-to-end vs the 2e-2 gate,
    bit-deterministic) and cuts 1/8 of layer-1 PE time
  - gate logits computed in [B, n_exp] orientation; the fp8 dims use the
    exact hi+lo pair (all 4 lo slices) against bf16 gate weights, so
    gates stay accurate; softmax along the free dim on ScalarE + DVE
  - each expert [B,E] tile is accumulated into per-task + shared-pool f32
    accumulators (one merged SBUF tile, so multi-acc flushes can be one
    DMA) with one fused scalar_tensor_tensor (acc = s*g + acc) on DVE
  - layer 2 is software-pipelined ONE GRANULE behind layer 1, so the last
    h-chunk's Act eviction never stalls PE at granule boundaries
  - the src-3 input + expert 12's w8 are host-packed into a "boot" tensor
    in exact consumption order (one ~400KB DMA per slice): the DMA
    prologue start is bounded by HWDGE's 625ns/DMA serialization, not by
    small-piece latency. A Pool-memset-gated warmup matmul chain precedes
    the stream so the cost model's PE p-state ramp is at full clock when
    the real matmuls arrive
  - the four shared experts' granule-1 L1s run back-to-back in the
    preamble (they all consume the same resident boot x), so weights and
    b-halves stream in with ~2x slack against their need times
  - per-phase DMA discipline: the next src's two 1MB x8 loads are
    bracketed by weight prefetches for the current and next phase's
    experts; gates for src t+1 are emitted at the end of phase t so they
    never wait on x8 arrival
  - the last expert runs progressively finer granules with PER-B-TILE
    merged acc2+acc3 flushes (~2MB of final output must pipeline behind
    the combine rate rather than bunch after the final matmul), with the
    very last b-tile's two flushes split across the SP and Act HWDGE
    queues
"""

import numpy as np
import ml_dtypes

import concourse.bass as bass
import concourse.mybir as mybir
from concourse.tile import TileContext
from concourse.bass_utils import run_bass_kernel_spmd

BF16 = ml_dtypes.bfloat16
F8 = ml_dtypes.float8_e4m3fn

# Problem shapes (hardcoded per spec)
T, B, D, H, E = 3, 8192, 1024, 512, 256
S, NS = 4, 4
NCORES = 8
BL = B // NCORES          # per-core batch rows (1024)
NBT = BL // 128           # b-tiles of 128 per core (8)
KB = 0                    # bf16 contraction chunks (all-fp8 layer 1)
DSPL = 0                  # first fp8 dim
NSL = 4                   # fp8 DoubleRow slices (256 dims each: 2 k-tiles)
NSL_LO = 3                # lo-correction slices in L1 (slice 3's x-residual is
                          # dropped: +0.9e-2 rel err in quadrature, -1/8 of L1
                          # PE time; gates still consume the exact hi+lo pair)
KH = H // 128             # contraction chunks for layer 2 (4)
NEXP = T * S + NS         # 16 experts total
BN = 512                  # layer-1 moving free-dim chunk (1 PSUM bank of f32)

TRACE = False             # test harness sets kernel.TRACE = True for profiling
LAST_EXEC_NS = None

_CACHE = {}

# this walrus build rejects instructions carrying more than one semaphore wait
# condition ("Too many sync wait commands" in CoreV3 setupSyncWait; observed on
# Drain with 2+ and TensorTensor with 2), but Tile's sem-assigner and tail
# drain emit up to ~11 on one instruction
DRAIN_KEEP = 1
OTHER_KEEP = 1


def _split_excess_waits(nc):
    """Move overflow sem-waits onto same-engine NOPs inserted just before the
    overloaded instruction. Waiting earlier on the same engine preserves the
    ordering guarantee the wait provides."""
    n_split = 0
    for f in nc.m.functions:
        for bb in f.blocks:
            insts = bb.instructions
            need = False
            for i in insts:
                si = i.sync_info
                if si and si.on_wait and len(si.on_wait) > (
                    DRAIN_KEEP if isinstance(i, mybir.InstDrain) else OTHER_KEEP
                ):
                    need = True
                    break
            if not need:
                continue
            new_insts = []
            for inst in insts:
                si = inst.sync_info
                waits = list(si.on_wait) if si and si.on_wait else []
                # DMA-queue sems resolve last (the tail flushes): keep them
                # on the instruction / last NoOps so the other waits process
                # during the final transfer instead of after its sem fires
                waits.sort(key=lambda w: (
                    'DMA' in (w.ant_name or ''),
                    tuple(-ord(c) for c in (w.ant_name or '')),
                ))
                keep = DRAIN_KEEP if isinstance(inst, mybir.InstDrain) else OTHER_KEEP
                if len(waits) > keep:
                    overflow = waits[: len(waits) - keep]
                    si.on_wait = waits[len(waits) - keep :]
                    for k, w in enumerate(overflow):
                        nop = mybir.InstNoOp(
                            name=f"{inst.name}-wsplit{k}", ins=[], outs=[]
                        )
                        nop.engine = inst.engine
                        nop.sync_info = mybir.SyncInfo(on_wait=[w], on_update=[])
                        new_insts.append(nop)
                        n_split += 1
                new_insts.append(inst)
            bb.instructions = new_insts
    return n_split


def _check_read_before_write(nc):
    """Emission-order lint: an on-chip tile read before any write means Tile
    will schedule the consumer against uninitialized memory."""
    import sys

    written = set()
    flagged = set()
    for f in nc.m.functions:
        for bb in f.blocks:
            for inst in bb.instructions:
                for arg in inst.ins:
                    t = getattr(getattr(arg, "bass_ap", None), "tensor", None)
                    name = getattr(t, "name", None)
                    if name and name not in written and name not in flagged:
                        space = getattr(t, "space", None)
                        if str(space) in ("MemorySpace.SBUF", "MemorySpace.PSUM"):
                            flagged.add(name)
                            print(
                                f"WARNING: {inst.name} reads {name} before any "
                                f"write (emission order)",
                                file=sys.stderr,
                            )
                for arg in inst.outs:
                    t = getattr(getattr(arg, "bass_ap", None), "tensor", None)
                    name = getattr(t, "name", None)
                    if name:
                        written.add(name)


def _build_program(with_b2=False, split_waits=True):
    f32 = mybir.dt.float32
    bf16 = mybir.dt.bfloat16
    fp8 = mybir.dt.float8e4
    relu = mybir.ActivationFunctionType.Relu
    expf = mybir.ActivationFunctionType.Exp
    mult = mybir.AluOpType.mult
    add = mybir.AluOpType.add
    DR = mybir.MatmulPerfMode.DoubleRow

    nc = bass.Bass()
    xT = (nc.dram_tensor("xT", [4, DSPL, BL], bf16, kind="ExternalInput")
          if KB else None)
    x8h = nc.dram_tensor("x8h", [3, 128, NSL * 2 * BL], fp8, kind="ExternalInput")
    x8l = nc.dram_tensor("x8l", [3, 128, NSL * 2 * BL], fp8, kind="ExternalInput")
    # boot tensor: src-3 (shared input) x pieces packed host-side WITH expert
    # 12's w8 in exact consumption order, so the DMA prologue is one ~400KB
    # DMA per slice instead of 3 small ones (HWDGE's 625ns/DMA serialization
    # was delivering slices slower than PE eats them).
    #   A-region, per slice sl: [w8[12]-sl | xh3-sl rows 0:512 | xl3-sl 0:512]
    #   B-region (base 12288), per sl: [xh3-sl rows 512:1024 | xl3-sl same]
    boot = nc.dram_tensor("boot", [128, 4 * 3072 + NSL * 2 * BL], fp8,
                          kind="ExternalInput")
    w1 = (nc.dram_tensor("w1", [NEXP, DSPL, H], bf16, kind="ExternalInput")
          if KB else None)
    w8 = nc.dram_tensor("w8", [NEXP, 128, NSL * 2 * H], fp8, kind="ExternalInput")
    w2 = nc.dram_tensor("w2", [NEXP, H, E], bf16, kind="ExternalInput")
    b1 = nc.dram_tensor("b1", [128, NEXP * KH], f32, kind="ExternalInput")
    wg = nc.dram_tensor("wg", [128, 4 * 8 * 16], bf16, kind="ExternalInput")
    bg = nc.dram_tensor("bg", [128, 4 * 16], f32, kind="ExternalInput")
    if with_b2:
        b2bc = nc.dram_tensor("b2bc", [128, NEXP * E], f32, kind="ExternalInput")
    out = nc.dram_tensor("out", [4, BL, E], f32, kind="ExternalOutput")

    with TileContext(nc) as tc:
        with (
            tc.tile_pool(name="const", bufs=1) as constp,
            tc.tile_pool(name="xp", bufs=1) as xp,
            tc.tile_pool(name="x8p", bufs=1) as x8p,
            tc.tile_pool(name="accp", bufs=1) as accp,
            tc.tile_pool(name="w1p", bufs=4) as w1p,
            tc.tile_pool(name="w8p", bufs=4) as w8p,
            tc.tile_pool(name="w2p", bufs=4) as w2p,
            tc.tile_pool(name="hp", bufs=3) as hp,
            tc.tile_pool(name="sp", bufs=6) as sp,
            tc.tile_pool(name="gp", bufs=4) as gp,
            tc.tile_pool(name="shp", bufs=8) as shp,
            tc.tile_pool(name="psh", bufs=4, space="PSUM") as psh_pool,
            tc.tile_pool(name="pss", bufs=4, space="PSUM") as pss_pool,
        ):
            wg_sb = constp.tile([128, 4 * 8 * 16], bf16)
            bg_sb = constp.tile([128, 4 * 16], f32)
            ebg_sb = constp.tile([128, 4 * 16], f32)
            b1_sb = constp.tile([128, NEXP * KH], f32)
            if with_b2:
                b2_sb = constp.tile([128, NEXP * E], f32)

            xt_sb = ([
                xp.tile([128, KB * BL], bf16, name=f"xt{src}") for src in range(4)
            ] if KB else None)
            x8h_sb = [
                x8p.tile([128, NSL * 2 * BL], fp8, name=f"x8h{src}")
                for src in range(3)
            ]
            x8l_sb = [
                x8p.tile([128, NSL * 2 * BL], fp8, name=f"x8l{src}")
                for src in range(3)
            ]
            boot_sb = x8p.tile([128, 4 * 3072 + NSL * 2 * BL], fp8, name="boot")

            def load_xt(src, half=None):
                if not KB:
                    return
                # one DMA per 2-chunk half: HWDGE descriptor processing is a
                # shared ~625ns/DMA serial resource, so fewer+bigger wins
                for h in ([half] if half is not None else (0, 1)):
                    c0 = h * (KB // 2)
                    nc.sync.dma_start(
                        out=xt_sb[src][:, c0 * BL : (c0 + KB // 2) * BL].rearrange(
                            "p (c b) -> p c b", c=KB // 2
                        ),
                        in_=xT[src][c0 * 128 : (c0 + KB // 2) * 128, :].rearrange(
                            "(c p) b -> p c b", p=128
                        ),
                    )

            def load_x8(src):
                nc.sync.dma_start(out=x8h_sb[src], in_=x8h[src])
                nc.sync.dma_start(out=x8l_sb[src], in_=x8l[src])

            def x8_base(s, hi, half):
                # boot column base of the [kt, 512] piece for (slice, hi/lo,
                # b-half)
                if half == 0:
                    return s * 3072 + 1024 + (0 if hi else 1024)
                return 4 * 3072 + s * 2048 + (0 if hi else 1024)

            def x8_slice(src, hi, s, b0, b1_):
                # [128, kt=2, b] access pattern for DoubleRow rhs
                if src < 3:
                    x8sb = x8h_sb[src] if hi else x8l_sb[src]
                    return x8sb[:, s * 2 * BL : (s + 1) * 2 * BL].rearrange(
                        "p (kt b) -> p kt b", kt=2
                    )[:, :, b0:b1_]
                half = b0 // BN
                assert (b1_ - 1) // BN == half
                base = x8_base(s, hi, half)
                return boot_sb[:, base : base + 1024].rearrange(
                    "p (kt b) -> p kt b", kt=2
                )[:, :, b0 - half * BN : b1_ - half * BN]

            def x8_col(src, hi, s, kt, b0, n=128):
                # [128, n] single-ktile column range (gate lhsT pieces)
                if src < 3:
                    x8sb = x8h_sb[src] if hi else x8l_sb[src]
                    c0 = s * 2 * BL + kt * BL + b0
                    return x8sb[:, c0 : c0 + n]
                half = b0 // BN
                c0 = x8_base(s, hi, half) + kt * BN + b0 - half * BN
                return boot_sb[:, c0 : c0 + n]

            def w8_slice(w8sb, s, hc):
                # [128, kt=2, 128] access pattern for DoubleRow lhsT; expert
                # 12's w8 lives in the boot tile (sentinel "boot")
                if isinstance(w8sb, str):
                    return boot_sb[:, s * 3072 : s * 3072 + 1024].rearrange(
                        "p (kt h) -> p kt h", kt=2
                    )[:, :, hc * 128 : (hc + 1) * 128]
                return w8sb[:, s * 2 * H : (s + 1) * 2 * H].rearrange(
                    "p (kt h) -> p kt h", kt=2
                )[:, :, hc * 128 : (hc + 1) * 128]

            # one merged accumulator tile for all 4 outputs: lets a single
            # flush DMA cover several accumulators (3D AP over t, b, f), which
            # matters in the tail where HWDGE's 625ns/DMA serializes flushes
            acc_all = accp.tile([128, 4 * NBT * E], f32, name="acc")

            def acc_ap(t, bt, nb=1):
                c0 = (t * NBT + bt) * E
                return acc_all[:, c0 : c0 + nb * E]

            gate_sb = [
                constp.tile([128, NBT * 16], f32, name=f"gate{s}") for s in range(4)
            ]
            written = set()  # (acc_idx, bt) already initialized

            def emit_gates(src):
                wexp = 8 if src < 3 else 16
                # one psg region holds all NBT b-tiles of this gate set as a
                # single accumulation group. Blocks 0..3 are the bf16 x
                # chunks; blocks 4..7 are the fp8 (slice, ktile) dims, fed
                # with the exact hi+lo pair against bf16 gate weights (the
                # wg blocks carry the matching x4 scale).
                psg = pss_pool.tile([128, NBT * wexp], f32, name="psg", tag="ps_s")
                n_mm = (KB + NSL * 2 * 2) * NBT
                i_mm = 0
                for blk in range(8):
                    if blk < KB:
                        lhs_list = [
                            xt_sb[src][:, blk * BL + bt * 128 : blk * BL + bt * 128 + 128]
                            for bt in range(NBT)
                        ] * 1
                        lhs_iter = [(bt, lhs_list[bt]) for bt in range(NBT)]
                    else:
                        s, kt = divmod(blk - KB, 2)
                        lhs_iter = []
                        for hi in (True, False):
                            for bt in range(NBT):
                                lhs_iter.append(
                                    (bt, x8_col(src, hi, s, kt, bt * 128))
                                )
                    for bt, lhs in lhs_iter:
                        nc.tensor.matmul(
                            psg[:, bt * wexp : bt * wexp + wexp],
                            lhsT=lhs,
                            rhs=wg_sb[:, (src * 8 + blk) * 16 : (src * 8 + blk) * 16 + wexp],
                            start=(i_mm == 0),
                            stop=(i_mm == n_mm - 1),
                        )
                        i_mm += 1
                # psg eviction on DVE: at phase boundaries Act is busy with
                # the neighboring experts' L1/L2 evictions, and a serial Act
                # burst here stalled the next expert's first matmuls
                # single whole-tile exp on Act (vs 8 per-bt exp+accum ops):
                # the serial per-bt Act chain here used to queue ahead of the
                # neighboring experts' L1 evictions and starve PE of psum
                # slots at every gate emission; per-bt sums move to DVE.
                # exp(l + b) = exp(l)*exp(b): the gate bias folds in
                # multiplicatively via the precomputed ebg tile
                logits = gp.tile([128, NBT * wexp], f32, tag="logits")
                nc.scalar.copy(logits, psg)
                gexp = gp.tile([128, NBT * wexp], f32, tag="gexp")
                nc.scalar.activation(gexp, logits, expf)
                for bt in range(NBT):
                    g_ap = gate_sb[src][:, bt * 16 : bt * 16 + wexp]
                    ge = gp.tile([128, 16], f32, tag="logit")
                    nc.vector.tensor_mul(
                        ge[:, :wexp],
                        gexp[:, bt * wexp : bt * wexp + wexp],
                        ebg_sb[:, src * 16 : src * 16 + wexp],
                    )
                    ssum = gp.tile([128, 1], f32, tag="ssum")
                    nc.vector.tensor_reduce(
                        ssum, ge[:, :wexp], axis=mybir.AxisListType.X, op=add,
                    )
                    rsum = gp.tile([128, 1], f32, tag="rsum")
                    nc.vector.reciprocal(rsum, ssum)
                    nc.vector.tensor_scalar_mul(g_ap, ge[:, :wexp], rsum)

            # SBUF staging for shared experts computed before the task gates
            # exist: their [B,E] tiles wait here until the gates are ready
            stage = {}

            def load_w1(e):
                if not KB:
                    return None
                w1_sb = w1p.tile([128, KB * H], bf16, name="w1_sb", tag="w1_sb")
                nc.sync.dma_start(
                    out=w1_sb.rearrange("p (c h) -> p c h", c=KB),
                    in_=w1[e].rearrange("(c p) h -> p c h", p=128),
                )
                return w1_sb

            def load_w8(e, pieces=1):
                w8_sb = w8p.tile([128, NSL * 2 * H], fp8, name="w8_sb", tag="w8_sb")
                step = NSL * 2 * H // pieces
                for q in range(pieces):
                    nc.sync.dma_start(
                        out=w8_sb[:, q * step : (q + 1) * step],
                        in_=w8[e][:, q * step : (q + 1) * step],
                    )
                return w8_sb

            def load_w2(e):
                w2_sb = w2p.tile([128, KH * E], bf16, name="w2_sb", tag="w2_sb")
                nc.sync.dma_start(
                    out=w2_sb.rearrange("p (c f) -> p c f", c=KH),
                    in_=w2[e].rearrange("(c p) f -> p c f", p=128),
                )
                return w2_sb

            def contribs_of(e):
                if e < T * S:
                    t, s = divmod(e, S)
                    return [(3, t * S + s), (t, s)]
                jsh = e - T * S
                return [(t, S + jsh) for t in range(T)] + [(3, T * S + jsh)]

            def emit_contrib(gset, col, bt, src_tile):
                # combines live on DVE: the only engine whose ISA has the
                # fused scalar_tensor_tensor (acc = s*g + acc); Pool cannot
                # even read PSUM on real TRN2
                g = gate_sb[gset][:, bt * 16 + col : bt * 16 + col + 1]
                a = acc_ap(gset, bt)
                if (gset, bt) not in written:
                    written.add((gset, bt))
                    nc.vector.tensor_scalar_mul(a, src_tile, g)
                else:
                    nc.vector.scalar_tensor_tensor(
                        a, src_tile, g, a, op0=mult, op1=add
                    )

            def emit_contribs(e, bt, src_tile):
                for gset, col in contribs_of(e):
                    emit_contrib(gset, col, bt, src_tile)

            def flush(ts, row0, nrows, eng=None):
                """Flush rows [row0, row0+nrows) of accumulators ts to DRAM.
                DMA APs allow at most 3 dims, so a multi-acc flush in ONE DMA
                only works for a single b-tile ([p, t, f]); otherwise one DMA
                per accumulator ([p, b, f])."""
                nb = nrows // 128
                b0 = row0 // 128
                t0 = ts[0]
                nt = len(ts)
                if nt > 1 and nb == 1:
                    assert list(ts) == list(range(t0, t0 + nt))
                    (eng or nc.sync).dma_start(
                        out=out[t0 : t0 + nt, row0 : row0 + nrows, :].rearrange(
                            "t (b p) f -> p (t b) f", p=128
                        ),
                        in_=acc_all.rearrange("p (t b f) -> p t b f", t=4, b=NBT)[
                            :, t0 : t0 + nt, b0, :
                        ],
                    )
                    return
                for t in ts:
                    (eng or nc.sync).dma_start(
                        out=out[t][row0 : row0 + nrows, :].rearrange(
                            "(b p) f -> p b f", p=128
                        ),
                        in_=acc_all[:, (t * NBT + b0) * E : (t * NBT + b0 + nb) * E]
                        .rearrange("p (b f) -> p b f", b=nb),
                    )

            def emit_l1_group(ps_h, w1_sb, w8_sb, src, hc, off, W):
                """One layer-1 accumulation group: 4 bf16 chunks + 2 fp8
                DoubleRow slices x (hi, lo)."""
                for c in range(KB):
                    nc.tensor.matmul(
                        ps_h,
                        lhsT=w1_sb[:, c * H + hc * 128 : c * H + hc * 128 + 128],
                        rhs=xt_sb[src][:, c * BL + off : c * BL + off + W],
                        start=(c == 0),
                        stop=False,
                    )
                passes = [(s, hi) for s in range(NSL)
                          for hi in ((True,) if s >= NSL_LO else (True, False))]
                for i, (s, hi) in enumerate(passes):
                    nc.tensor.matmul(
                        ps_h,
                        lhsT=w8_slice(w8_sb, s, hc),
                        rhs=x8_slice(src, hi, s, off, off + W),
                        start=(KB == 0 and i == 0),
                        stop=(i == len(passes) - 1),
                        perf_mode=DR,
                    )

            DEFAULT_BN = [(0, BN), (BN, BN)]
            # last expert: progressively finer granules so only ONE b-tile's
            # evict+combine+flush chain drains after the last matmul, and the
            # ~1.5MB of final acc2+acc3 flushes start as early as possible
            # (the DMA device is serial at ~360GB/s)
            TAIL_BN = [(0, BN), (BN, BN // 2), (768, 256)]

            # Software pipeline: each granule's layer 2 is emitted AFTER the
            # NEXT granule's layer-1 groups, so the last h-chunk's Act
            # eviction (~700ns) is hidden under ~3us of L1 instead of
            # stalling PE at every granule boundary.
            pending_l2 = []

            def drain_l2():
                while pending_l2:
                    pending_l2.pop(0)()

            def emit_expert(e, src, finalize, defer=False, extra_per_bt=None,
                            w1_pre=None, w8_pre=None, h_pre=None, w2_pre=None,
                            fine_tail=False, granules=None):
                w1_sb = w1_pre if w1_pre is not None else load_w1(e)
                w8_sb = w8_pre if w8_pre is not None else load_w8(e)
                w2_sb = w2_pre if w2_pre is not None else load_w2(e)

                def make_l2(h_sb, off, W):
                    def l2():
                        for j in range(W // 128):
                            bt = off // 128 + j
                            ps_s = pss_pool.tile([128, E], f32, name="ps_s",
                                                 tag="ps_s")
                            for hc in range(KH):
                                nc.tensor.matmul(
                                    ps_s,
                                    lhsT=h_sb[:, hc * W + j * 128 :
                                              hc * W + j * 128 + 128],
                                    rhs=w2_sb[:, hc * E : (hc + 1) * E],
                                    start=(hc == 0),
                                    stop=(hc == KH - 1),
                                )
                            if with_b2:
                                nc.vector.tensor_add(
                                    ps_s, ps_s, b2_sb[:, e * E : (e + 1) * E]
                                )
                            if defer:
                                st = shp.tile([128, E], bf16, name=f"st{e}",
                                              tag=f"st{e}")
                                nc.scalar.activation(st, ps_s, relu)
                                stage[(e, bt)] = st
                                if extra_per_bt is not None:
                                    extra_per_bt(bt)
                            else:
                                s_sb = sp.tile([128, E], bf16, name="s_sb",
                                               tag="s_sb")
                                # all tail evictions on Act: the final
                                # DVE chain stays combines-only
                                nc.scalar.activation(s_sb, ps_s, relu)
                                # deferred shared-expert combines first: they
                                # only need staged tiles + gates, so they
                                # never sit on the critical tail chain
                                if extra_per_bt is not None:
                                    extra_per_bt(bt)
                                emit_contribs(e, bt, s_sb)

                            # flush finished accumulator rows to DRAM as soon
                            # as their last contribution lands. The last
                            # expert flushes PER B-TILE (one merged acc2+acc3
                            # DMA each): ~2MB of final flushes must pipeline
                            # behind the combine rate, not bunch at the end.
                            if finalize:
                                if fine_tail:
                                    if bt == NBT - 1:
                                        # per-acc on separate queues: acc3's
                                        # combine lands one combine earlier
                                        flush([3], bt * 128, 128)
                                        flush([2], bt * 128, 128,
                                              eng=nc.scalar)
                                    else:
                                        flush(finalize, bt * 128, 128)
                                elif j == W // 128 - 1:
                                    flush(finalize, off, W)
                    return l2

                if granules is None:
                    granules = TAIL_BN if fine_tail else DEFAULT_BN
                for off, W in granules:
                    if h_pre is not None and off == 0:
                        h_sb = h_pre
                    else:
                        h_sb = hp.tile([128, KH * W], bf16, name="h_sb", tag="h_sb")
                        for hc in range(KH):
                            ps_h = psh_pool.tile([128, W], f32, name="ps_h", tag="ps_h")
                            emit_l1_group(ps_h, w1_sb, w8_sb, src, hc, off, W)
                            nc.scalar.activation(
                                h_sb[:, hc * W : (hc + 1) * W],
                                ps_h,
                                relu,
                                bias=b1_sb[:, e * KH + hc : e * KH + hc + 1],
                            )
                        drain_l2()
                    pending_l2.append(make_l2(h_sb, off, W))

            # Emission = per-engine program order (modulo the Tile list
            # scheduler). Shared experts first: they need no gates at compute
            # time (combine deferred via SBUF staging), then gates(3), the
            # remaining shared experts, then task phases 0/1/2 with the
            # deferred shared-pool contributions interleaved per b-tile.
            finalize_at = {3: [0], 7: [1], 11: [2, 3]}

            # DMA prologue: per-slice boot DMAs deliver (w8[12]-slice, x-hi,
            # x-lo) in exact consumption order at ~1.1us each (one HWDGE slot
            # per slice), then the small constants, then the b-halves.
            w1_12 = None
            # boot slices take the first HWDGE slots (the start is bounded by
            # HWDGE+transfer serialization); the PE warmup rides a Pool
            # memset instead of a DMA (the cost model's p-state ramp would
            # otherwise run the first ~3us of matmuls at 1.2GHz / 0.65GHz)
            wt_sb = constp.tile([128, 64], fp8, name="wt_sb")
            nc.gpsimd.memset(wt_sb[:], 0.0)
            for sl in range(NSL):
                nc.sync.dma_start(
                    out=boot_sb[:, sl * 3072 : (sl + 1) * 3072],
                    in_=boot[:, sl * 3072 : (sl + 1) * 3072],
                )
            nc.sync.dma_start(out=b1_sb, in_=b1[:, :])
            def load_bhalf(sl):
                c0 = 4 * 3072 + sl * 2048
                nc.sync.dma_start(
                    out=boot_sb[:, c0 : c0 + 2048],
                    in_=boot[:, c0 : c0 + 2048],
                )
            # granule-interleaved preamble DMA stream, ordered by need time:
            # all four shared experts consume the SAME resident boot x, so
            # their granule-1 L1s run back-to-back while weights/b-halves
            # stream in with slack
            w8_13 = load_w8(13, pieces=4)
            w2_12 = load_w2(12)
            load_bhalf(0)
            load_bhalf(1)
            nc.sync.dma_start(out=wg_sb, in_=wg[:, :])
            nc.sync.dma_start(out=bg_sb, in_=bg[:, :])
            nc.scalar.activation(ebg_sb, bg_sb, expf)
            if with_b2:
                nc.sync.dma_start(out=b2_sb, in_=b2bc[:, :])
            w8_14 = load_w8(14, pieces=2)
            w2_13 = load_w2(13)
            load_bhalf(2)
            load_bhalf(3)
            w8_15 = load_w8(15)
            w2_14 = load_w2(14)
            w2_15 = load_w2(15)
            load_x8(0)

            # PE prologue: expert 12's first-half layer-1 runs c-outer across
            # all 4 h-chunk PSUM banks (3 psh banks + 1 pss bank) so PE
            # consumes each boot slice as it lands; evictions split Act/DVE
            # so layer 2 isn't gated on a serial Act eviction burst.
            h12 = hp.tile([128, KH * BN], bf16, name="h12", bufs=1)
            # p-state warmup chain on the memset tile (output never read)
            psw = pss_pool.tile([64, 1], f32, name="psw", tag="ps_s")
            for i in range(4):
                nc.tensor.matmul(psw, lhsT=wt_sb, rhs=wt_sb[:, :1],
                                 start=(i == 0), stop=(i == 3))
            # prologue h-chunk banks split 2+2 across the psh/pss pools so
            # e13-g1's first two L1 groups get WAR-free psh banks
            ph = [
                psh_pool.tile([128, BN], f32, name=f"ph{hc}", tag="ps_h")
                for hc in range(2)
            ] + [
                pss_pool.tile([128, BN], f32, name=f"ph{hc}", tag="ps_s")
                for hc in range(2, 4)
            ]
            pro_passes = [(s, hi) for s in range(NSL)
                          for hi in ((True,) if s >= NSL_LO else (True, False))]
            for i, (s, hi) in enumerate(pro_passes):
                first = i == 0
                last = i == len(pro_passes) - 1
                for hc in range(4):
                    nc.tensor.matmul(
                        ph[hc],
                        lhsT=w8_slice("boot", s, hc),
                        rhs=x8_slice(3, hi, s, 0, BN),
                        start=first,
                        stop=last,
                        perf_mode=DR,
                    )
            for hc in range(4):
                if hc % 2:
                    nc.vector.tensor_scalar(
                        h12[:, hc * BN : (hc + 1) * BN], ph[hc],
                        b1_sb[:, 12 * KH + hc : 12 * KH + hc + 1],
                        0.0, op0=add, op1=mybir.AluOpType.max,
                    )
                else:
                    nc.scalar.activation(
                        h12[:, hc * BN : (hc + 1) * BN], ph[hc], relu,
                        bias=b1_sb[:, 12 * KH + hc : 12 * KH + hc + 1],
                    )

            pre_w = {12: ("boot", w2_12), 13: (w8_13, w2_13),
                     14: (w8_14, w2_14), 15: (w8_15, w2_15)}
            for ee in (12, 13, 14, 15):
                emit_expert(ee, 3, [], defer=True, w1_pre=w1_12,
                            w8_pre=pre_w[ee][0], w2_pre=pre_w[ee][1],
                            h_pre=h12 if ee == 12 else None,
                            granules=[(0, BN)])
            emit_gates(3)
            for ee in (12, 13, 14, 15):
                emit_expert(ee, 3, [], defer=True, w1_pre=w1_12,
                            w8_pre=pre_w[ee][0], w2_pre=pre_w[ee][1],
                            granules=[(BN, BN)])
            emit_gates(0)

            # deferred shared-expert combines, spread across the task phases:
            # st_{12+k} -> acc_t runs during phase t's k-th expert (so
            # gates(1)/gates(2) aren't needed until their own phase), and
            # st_{12+k} -> acc3 runs during phase 0.
            def make_hook(pairs):
                def hook(bt):
                    for k, gset in pairs:
                        col = S + k if gset < 3 else T * S + k
                        emit_contrib(gset, col, bt, stage[(12 + k, bt)])
                return hook

            # hook distribution: the finalizing expert of each phase (e3/e7/
            # e11) carries no deferred combines so its tail chain stays
            # short, and phase 2 (ending at the kernel tail) carries only ONE
            # hook per expert — DVE's per-expert combine budget is ~3.5 ops/
            # b-tile, and a 4-op expert right before e11 was backlogging the
            # final combines by ~1us
            PH_HOOKS = {
                0: [[(0, 0), (1, 0)], [(2, 0), (2, 3)], [(3, 0), (3, 3)], []],
                1: [[(0, 1), (0, 3)], [(1, 1), (1, 3)], [(2, 1), (3, 1)], []],
                2: [[(0, 2), (3, 2)], [(1, 2)], [(2, 2)], []],
            }
            # DMA-order discipline inside the task phases: before the next
            # src's two 1MB x8 DMAs hit the (serialized) DMA path, prefetch
            # this phase's last expert's weights; right after them, prefetch
            # the next phase's first expert's weights. Gates for src t+1 move
            # to the end of phase t so their matmuls never wait on x8 arrival.
            pre = {}
            for ph, src in ((0, 0), (1, 1), (2, 2)):
                for k, e in enumerate(range(ph * 4, ph * 4 + 4)):
                    if k == 3 and ph < 2:
                        pre[e] = (load_w8(e), load_w2(e))
                        load_x8(src + 1)
                        pre[e + 1] = (load_w8(e + 1), load_w2(e + 1))
                    w8_pre, w2_pre = pre.pop(e, (None, None))
                    emit_expert(e, src, finalize_at.get(e, []),
                                extra_per_bt=make_hook(PH_HOOKS[ph][k]),
                                w8_pre=w8_pre, w2_pre=w2_pre,
                                fine_tail=(e == 11))
                if ph < 2:
                    emit_gates(src + 1)
            drain_l2()

    _check_read_before_write(nc)
    if split_waits:
        _split_excess_waits(nc)
    return nc


def _prep_shared(W_spec1, b_spec1, W_spec2, b_spec2, W_sh1, b_sh1, W_sh2, b_sh2,
                 W_gate, b_gate, W_gate_sh, b_gate_sh, with_b2):
    """Host-side prep of the replicated (per-core-identical) tensors."""
    W1_all = np.concatenate([W_spec1, W_sh1], axis=0).astype(np.float32)
    w1 = (np.ascontiguousarray(W1_all[:, :DSPL, :].astype(BF16))
          if KB else None)
    # fp8 upper-half weights, x4 pre-scale (pairs with the x/4 activations),
    # layout [e][p][(slice, ktile, h)] for the DoubleRow lhsT
    w8 = np.ascontiguousarray(
        (W1_all[:, DSPL:, :] * 4.0)
        .astype(F8)
        .reshape(NEXP, NSL, 2, 128, H)
        .transpose(0, 3, 1, 2, 4)
        .reshape(NEXP, 128, NSL * 2 * H)
    )
    w2 = np.ascontiguousarray(
        np.concatenate([W_spec2, W_sh2], axis=0).astype(BF16)
    )
    # layer-1 biases, batched: [128, e*KH + hc] (partition = h within chunk)
    b1 = np.ascontiguousarray(
        np.concatenate([b_spec1, b_sh1], axis=0)
        .astype(np.float32)
        .reshape(NEXP, KH, 128)
        .transpose(2, 0, 1)
        .reshape(128, NEXP * KH)
    )
    # gate weights: [4 sets, D, 16] (task gates padded 8 -> 16 with zeros);
    # blocks 4..7 (dims 512..1023) are consumed against the /4-scaled fp8
    # activations, so they carry the compensating x4 scale
    wg_full = np.zeros((4, D, 16), np.float32)
    wg_full[:T, :, : S + NS] = W_gate
    wg_full[3] = W_gate_sh
    wg_blocks = wg_full.reshape(4, 8, 128, 16).copy()
    wg_blocks[:, KB:] *= 4.0
    wg = np.ascontiguousarray(
        wg_blocks.transpose(2, 0, 1, 3).reshape(128, 4 * 8 * 16)
    ).astype(BF16)
    bg_full = np.zeros((4, 16), np.float32)
    bg_full[:T, : S + NS] = b_gate
    bg_full[3] = b_gate_sh
    bg = np.ascontiguousarray(
        np.broadcast_to(bg_full[None], (128, 4, 16)).reshape(128, 64)
    )
    res = dict(w8=w8, w2=w2, b1=b1, wg=wg, bg=bg)
    if KB:
        res["w1"] = w1
    if with_b2:
        b2_full = np.concatenate([b_spec2, b_sh2], axis=0).astype(np.float32)
        res["b2bc"] = np.ascontiguousarray(
            np.broadcast_to(b2_full.reshape(1, NEXP * E), (128, NEXP * E))
        )
    return res


def kernel(x_tasks, x_shared, W_spec1, b_spec1, W_spec2, b_spec2,
           W_sh1, b_sh1, W_sh2, b_sh2, W_gate, b_gate, W_gate_sh, b_gate_sh):
    global LAST_EXEC_NS
    with_b2 = bool(np.any(np.asarray(b_spec2)) or np.any(np.asarray(b_sh2)))
    key = ("nc", with_b2)
    if key not in _CACHE:
        _CACHE[key] = _build_program(with_b2=with_b2)
    nc = _CACHE[key]
    _CACHE["nc"] = nc  # latest program, for the test harness's TimelineSim

    shared = _prep_shared(W_spec1, b_spec1, W_spec2, b_spec2, W_sh1, b_sh1,
                          W_sh2, b_sh2, W_gate, b_gate, W_gate_sh, b_gate_sh,
                          with_b2)

    x_tasks = np.asarray(x_tasks, np.float32)
    x_shared = np.asarray(x_shared, np.float32)

    in_maps = []
    for i in range(NCORES):
        sl = slice(i * BL, (i + 1) * BL)
        xt = np.empty((4, DSPL, BL), BF16) if KB else None
        xh = np.empty((3, 128, NSL * 2 * BL), F8)
        xl = np.empty((3, 128, NSL * 2 * BL), F8)
        boot = np.empty((128, 4 * 3072 + NSL * 2 * BL), F8)
        srcs = [x_tasks[0, sl], x_tasks[1, sl], x_tasks[2, sl], x_shared[sl]]
        for s_i, xsrc in enumerate(srcs):
            if KB:
                xt[s_i] = xsrc[:, :DSPL].T.astype(BF16)
            xs = (xsrc[:, DSPL:] / 4.0).astype(np.float32)   # [BL, 512]
            hi = xs.astype(F8)
            lo = (xs - hi.astype(np.float32)).astype(F8)
            # layout [p][(slice, ktile, b)]
            hi_r = hi.reshape(BL, NSL, 2, 128).transpose(3, 1, 2, 0)
            lo_r = lo.reshape(BL, NSL, 2, 128).transpose(3, 1, 2, 0)
            if s_i < 3:
                xh[s_i] = hi_r.reshape(128, NSL * 2 * BL)
                xl[s_i] = lo_r.reshape(128, NSL * 2 * BL)
            else:
                # boot pack: per slice [w8[12]-sl | x-hi a-half | x-lo
                # a-half], then b-halves per slice [hi | lo]
                for s in range(NSL):
                    c = s * 3072
                    boot[:, c : c + 1024] = shared["w8"][12][:, s * 2 * H :
                                                             (s + 1) * 2 * H]
                    boot[:, c + 1024 : c + 2048] = hi_r[:, s, :, :512].reshape(
                        128, 1024)
                    boot[:, c + 2048 : c + 3072] = lo_r[:, s, :, :512].reshape(
                        128, 1024)
                    cb = 4 * 3072 + s * 2048
                    boot[:, cb : cb + 1024] = hi_r[:, s, :, 512:].reshape(
                        128, 1024)
                    boot[:, cb + 1024 : cb + 2048] = lo_r[:, s, :, 512:].reshape(
                        128, 1024)
        m = {"x8h": xh, "x8l": xl, "boot": boot}
        if KB:
            m["xT"] = xt
        m.update(shared)
        in_maps.append(m)

    res = run_bass_kernel_spmd(nc, in_maps, core_ids=list(range(NCORES)), trace=TRACE)
    LAST_EXEC_NS = res.exec_time_ns

    full = np.empty((T + 1, B, E), np.float32)
    for i in range(NCORES):
        full[:, i * BL : (i + 1) * BL, :] = res.results[i]["out"]
    return full

